# revision 11
# baseline (speedup 1.0000x reference)
"""Trainium2 Bass kernel for nn_ConditionalEstimation.

Computes, for full inputs:
    context[i] = sum_{j,k} a[i,j,k] * y[j] * z[k]          (i in [0, 384))
    scores[n]  = (x[n, :] @ context) / (context[0] + 1e-8)

Sharding across 8 NeuronCores (SPMD, one NEFF):
    - a is sharded along its leading i axis: core c owns a[c*48:(c+1)*48].
      Each core computes its 48-element slice of `context`, then an
      AllGather assembles the full 384-vector on every core.
    - x_candidates is sharded along N: core c owns rows [c*8192, (c+1)*8192)
      and computes those scores (pure data parallel).

Per-core device algorithm (engines chosen so the kernel is DMA-bound):
    phase 1 (TensorE, float32r): per i, one DMA loads a[i] as [128, 3, 384]
        with partition p holding the three consecutive j-rows 3p..3p+2
        (contiguous 4.6KB per partition => few, large DMA packets). Three
        matmuls with y-permuted-to-match stationary columns contract j:
        u_i[k] = sum_j y[j] a[i, j, k]. ScalarE copies each PSUM row to a
        flat SBUF row; one SBUF->SBUF DMA reshapes u to [48, 384]; VectorE
        contracts k with z.
    AllGather(48 -> 384). All x DMAs are issued on Sync before anything
        that waits on the collective, so x streams during the AllGather.
    phase 2 (split): VectorE mult+reduce for TD row-tiles; TensorE matvec
        (float32r) over host-transposed x chunks for the rest.
"""

import os
import sys

import numpy as np

sys.path.insert(0, "/opt/trn_rl_repo")

import concourse.bacc as bacc
import concourse.mybir as mybir
import concourse.tile as tile
from concourse.bass_utils import run_bass_kernel_spmd

N, D = 65536, 384
NC = 8
ISH = D // NC            # 48 context rows per core
XSH = N // NC            # 8192 candidate rows per core
EPS = 1e-8
FP = mybir.dt.float32
FPR = mybir.dt.float32r  # fp32 bits, reduced-precision PE compute (1 cyc/row)
USE_FPR = os.environ.get("CC_KERNEL_FP32R", "1") == "1"

TD = 12                  # phase-2 DVE tiles (each covers 128 rows)
RD = 128 * TD            # rows handled by the DVE path (1536)
RP = XSH - RD            # rows handled by the PE path (6656)
PCH = 512                # PE path chunk width
NCH = RP // PCH          # PE path chunks (13)

_CACHE = {}
LAST_RESULT = None  # BassKernelResults of the most recent run (for test harness)


def _build():
    if "nc" in _CACHE:
        return _CACHE["nc"]

    nc = bacc.Bacc("TRN2", target_bir_lowering=False, debug=False, num_devices=NC)
    Alu = mybir.AluOpType

    # float32r: same fp32 bytes, PE streams 1 row/cycle instead of 4.
    MMT = FPR if USE_FPR else FP

    a_d = nc.dram_tensor("a_sh", [ISH, D, D], MMT, kind="ExternalInput")
    xd_d = nc.dram_tensor("x_dve", [RD, D], FP, kind="ExternalInput")
    xp_d = nc.dram_tensor("xT_pe", [D, RP], MMT, kind="ExternalInput")
    y_d = nc.dram_tensor("y", [D], MMT, kind="ExternalInput")
    z_d = nc.dram_tensor("z", [D], FP, kind="ExternalInput")
    o_d = nc.dram_tensor("scores_sh", [XSH], FP, kind="ExternalOutput")

    with tile.TileContext(nc) as tc:
        with (
            tc.tile_pool(name="const", bufs=1) as cst,
            tc.tile_pool(name="a", bufs=10) as a_pool,
            tc.tile_pool(name="xtp", bufs=8) as xt_pool,
            tc.tile_pool(name="scr", bufs=6) as scr_pool,
            tc.tile_pool(name="acc", bufs=1) as acc_pool,
            tc.tile_pool(name="ps", bufs=8, space="PSUM") as ps_pool,
            tc.tile_pool(name="so", bufs=4) as so_pool,
            tc.tile_pool(name="dram", bufs=1, space="DRAM") as dram_pool,
        ):
            # --- constants ---
            zb = cst.tile([128, D], FP)      # z broadcast across partitions
            nc.sync.dma_start(zb[:], z_d.ap().unsqueeze(0).partition_broadcast(128))
            # y permuted to match the a-tile layout: y3p[p, s] = y[3p + s]
            y3p = cst.tile([128, 3], MMT)
            nc.sync.dma_start(y3p[:], y_d.ap().rearrange("(p s) -> p s", s=3))

            # --- phase 1: u_i[k] = sum_j y[j] a[i, j, k]  (TensorE) ---
            # a[i] loaded as [p, s, k] = a[i, 3p+s, k]: 4.6KB contiguous per
            # partition -> 128 large DMA packets instead of 384 small ones.
            u_flat = acc_pool.tile([1, ISH * D], FP)
            for i in range(ISH):
                at = a_pool.tile([128, 3, D], MMT)
                nc.sync.dma_start(at[:], a_d.ap()[i].rearrange("(p s) k -> p s k", s=3))
                ups = ps_pool.tile([1, D], FP, tag="ps")
                for s in range(3):
                    nc.tensor.matmul(
                        ups[:],
                        y3p[:, s:s + 1],
                        at[:, s, :],
                        start=(s == 0),
                        stop=(s == 2),
                    )
                nc.scalar.copy(u_flat[:, i * D:(i + 1) * D], ups[:])

            # reshape u to [48, 384] (GpSimd so Sync keeps issuing x DMAs)
            u_mat = acc_pool.tile([ISH, D], FP)
            nc.gpsimd.dma_start(
                u_mat[:], u_flat[:].rearrange("p (i k) -> p i k", i=ISH)
            )
            uz = acc_pool.tile([ISH, D], FP)
            nc.vector.tensor_mul(uz[:], u_mat[:], zb[0:ISH, :])
            ctx48 = acc_pool.tile([ISH, 1], FP)
            nc.vector.tensor_reduce(
                ctx48[:], uz[:], axis=mybir.AxisListType.X, op=Alu.add
            )

            # --- AllGather the context slices (bounce DMA on GpSimd) ---
            cc_in = dram_pool.tile([ISH], FP)
            cc_out = dram_pool.tile([D], FP)
            nc.gpsimd.dma_start(cc_in[:], ctx48[:])
            nc.gpsimd.collective_compute(
                "AllGather",
                Alu.bypass,
                replica_groups=[list(range(NC))],
                ins=[cc_in.opt()],
                outs=[cc_out.opt()],
            )

            # --- x prefetch: issued on Sync BEFORE anything that waits on the
            # AllGather, so the 12.6MB of x streams during the collective.
            # All of x_dve in one DMA: partition p holds its TD consecutive rows.
            xall = cst.tile([128, TD, D], FP)
            nc.sync.dma_start(
                xall[:], xd_d.ap().rearrange("(p t) d -> p t d", t=TD)
            )
            xcs = []
            for c in range(NCH):
                xc = xt_pool.tile([128, 3, PCH], MMT)
                nc.sync.dma_start(
                    xc[:],
                    xp_d.ap()[:, c * PCH:(c + 1) * PCH].rearrange(
                        "(s p) q -> p s q", p=128
                    ),
                )
                xcs.append(xc)

            # --- post-AG context setup (Sync has nothing left but output DMAs)
            ctx_b = cst.tile([128, D], FP)   # full context, broadcast
            nc.sync.dma_start(ctx_b[:], cc_out[:].unsqueeze(0).partition_broadcast(128))
            den = cst.tile([128, 1], FP)     # context[0], broadcast
            nc.sync.dma_start(den[:], cc_out[0:1].unsqueeze(0).partition_broadcast(128))
            # normalized context, k-major columns, for the PE matvec path
            ctx3 = cst.tile([128, 3], FP)    # ctx3[p, s] = context[s*128 + p]
            nc.sync.dma_start(ctx3[:], cc_out[:].rearrange("(s p) -> p s", p=128))
            den_e = cst.tile([128, 1], FP)
            nc.vector.tensor_scalar_add(den_e[:], den[:], EPS)
            rec = cst.tile([128, 1], FP)
            nc.vector.reciprocal(rec[:], den_e[:])
            ctxn3 = cst.tile([128, 3], MMT)
            nc.vector.tensor_scalar_mul(ctxn3[:], ctx3[:], rec[:])

            # --- phase 2b (TensorE): rows [RD, 8192) via x^T chunks ---
            for c in range(NCH):
                sps = ps_pool.tile([1, PCH], FP, tag="ps")
                for kt in range(3):
                    nc.tensor.matmul(
                        sps[:],
                        ctxn3[:, kt:kt + 1],
                        xcs[c][:, kt, :],
                        start=(kt == 0),
                        stop=(kt == 2),
                    )
                so = so_pool.tile([1, PCH], FP)
                nc.scalar.copy(so[:], sps[:])
                nc.sync.dma_start(o_d.ap()[RD + c * PCH:RD + (c + 1) * PCH], so[:])

            # --- phase 2a (VectorE): rows [0, RD), n = p*TD + t ---
            scores = acc_pool.tile([128, TD], FP)
            for t in range(TD):
                scr = scr_pool.tile([128, D], FP)
                nc.vector.tensor_mul(scr[:], xall[:, t, :], ctx_b[:])
                nc.vector.tensor_reduce(
                    scores[:, t:t + 1], scr[:], axis=mybir.AxisListType.X, op=Alu.add
                )
            scoren = acc_pool.tile([128, TD], FP)
            nc.vector.tensor_scalar_mul(scoren[:], scores[:], rec[:])
            nc.sync.dma_start(
                o_d.ap()[0:RD].rearrange("(p t) -> p t", t=TD), scoren[:]
            )

    nc.compile()
    _CACHE["nc"] = nc
    return nc


def make_in_maps(x_candidates, y, z, a):
    x_candidates = np.ascontiguousarray(x_candidates, dtype=np.float32)
    y = np.ascontiguousarray(y, dtype=np.float32)
    z = np.ascontiguousarray(z, dtype=np.float32)
    a = np.ascontiguousarray(a, dtype=np.float32)
    in_maps = []
    for c in range(NC):
        x_sh = x_candidates[c * XSH:(c + 1) * XSH]
        in_maps.append({
            "a_sh": a[c * ISH:(c + 1) * ISH],
            "x_dve": x_sh[:RD],
            "xT_pe": np.ascontiguousarray(x_sh[RD:].T),
            "y": y,
            "z": z,
        })
    return in_maps


def kernel(x_candidates, y, z, a):
    global LAST_RESULT
    nc = _build()
    in_maps = make_in_maps(x_candidates, y, z, a)

    trace = os.environ.get("CC_KERNEL_TRACE", "0") == "1"
    try:
        res = run_bass_kernel_spmd(nc, in_maps, core_ids=list(range(NC)), trace=trace)
    except Exception:
        if not trace:
            raise
        # Trace post-processing can fail in minimal containers; results
        # are what matter — retry without tracing.
        res = run_bass_kernel_spmd(nc, in_maps, core_ids=list(range(NC)), trace=False)
    LAST_RESULT = res
    out = np.concatenate([res.results[c]["scores_sh"] for c in range(NC)])
    return np.ascontiguousarray(out, dtype=np.float32)
